# revision 7
# baseline (speedup 1.0000x reference)
"""CGC multi-gate MoE kernel for Trainium2 (8 NeuronCores, data-parallel over batch).

Problem: 12 experts (4 shared / 4 task0 / 4 task1), each a 2-layer ReLU MLP
D=1024 -> H1=512 -> H2=256, over B=4096 rows; 3 softmax gates combine expert
outputs into t0/t1/ts [B, 256].

Strategy: pure batch data-parallel (512 rows/core, no collectives). Host
pre-transposes x (so the contraction dim D lands on SBUF partitions) and
pre-tiles weights into the exact SBUF layout for fully-contiguous DMA. All
matmuls run in float32r (full PE rate for moving dim >= 256, ~1.5e-4 rel err).
Layer-1 output is kept transposed on-chip ([H1, B] layout) so layer 2 needs no
transpose and its output lands with B on partitions, ready for the per-row
gate-weighted combine on DVE. Biases: b1 fused into the layer-1 ReLU copy on
ACT (per-partition bias); b2 added in-PSUM via a K=1 ones-row matmul.
"""
import os
import sys

for _p in ("/opt/trn_rl_repo", "/root/.axon_site/_ro/trn_rl_repo"):
    if os.path.isdir(_p):
        if _p not in sys.path:
            sys.path.insert(0, _p)
        break

import numpy as np
from contextlib import ExitStack

import concourse.bass as bass
import concourse.mybir as mybir
import concourse.tile as tile
from concourse import bacc
from concourse.bass_utils import run_bass_kernel_spmd

B, D, H1, H2 = 4096, 1024, 512, 256
NE = 12          # 4 shared + 4 task0 + 4 task1
NCORES = 8
BC = B // NCORES # 512 rows per core
P = 128
KO1 = D // P     # 8 contraction chunks, layer 1
KO2 = H1 // P    # 4 contraction chunks, layer 2
MT1 = H1 // P    # 4 output M-tiles, layer 1 (H1 on partitions)
BT = BC // P     # 4 B-tiles per core
GW = 28          # gate logit widths, concatenated: 8 (g0) + 8 (g1) + 12 (gs)

F32 = mybir.dt.float32
F32R = mybir.dt.float32r
RELU = mybir.ActivationFunctionType.Relu
EXP = mybir.ActivationFunctionType.Exp
MULT = mybir.AluOpType.mult
ADD = mybir.AluOpType.add


def _build_program():
    nc = bacc.Bacc("TRN2", target_bir_lowering=False, debug=False, num_devices=NCORES)

    xst = nc.dram_tensor("xst", [P, KO1, BC], F32R, kind="ExternalInput")
    x0t = nc.dram_tensor("x0t", [P, KO1, BC], F32R, kind="ExternalInput")
    x1t = nc.dram_tensor("x1t", [P, KO1, BC], F32R, kind="ExternalInput")
    w1 = nc.dram_tensor("w1", [NE, P, KO1, H1], F32R, kind="ExternalInput")
    w2 = nc.dram_tensor("w2", [NE, P, KO2, H2], F32R, kind="ExternalInput")
    wg = nc.dram_tensor("wg", [P, KO1, GW], F32R, kind="ExternalInput")
    b1d = nc.dram_tensor("b1d", [P, NE, MT1], F32, kind="ExternalInput")
    b2d = nc.dram_tensor("b2d", [1, NE, H2], F32R, kind="ExternalInput")
    onesd = nc.dram_tensor("onesd", [1, P], F32R, kind="ExternalInput")
    t0d = nc.dram_tensor("t0d", [P, BT, H2], F32, kind="ExternalOutput")
    t1d = nc.dram_tensor("t1d", [P, BT, H2], F32, kind="ExternalOutput")
    tsd = nc.dram_tensor("tsd", [P, BT, H2], F32, kind="ExternalOutput")

    with tile.TileContext(nc) as tc, ExitStack() as ctx:
        const = ctx.enter_context(tc.tile_pool(name="const", bufs=1))
        xpool = ctx.enter_context(tc.tile_pool(name="xpool", bufs=1))
        w1pool = ctx.enter_context(tc.tile_pool(name="w1pool", bufs=2))
        w2pool = ctx.enter_context(tc.tile_pool(name="w2pool", bufs=2))
        hpool = ctx.enter_context(tc.tile_pool(name="hpool", bufs=2))
        opool = ctx.enter_context(tc.tile_pool(name="opool", bufs=3))
        gtmp = ctx.enter_context(tc.tile_pool(name="gtmp", bufs=2))
        l1ps = ctx.enter_context(tc.tile_pool(name="l1ps", bufs=4, space="PSUM"))
        l2ps = ctx.enter_context(tc.tile_pool(name="l2ps", bufs=4, space="PSUM"))

        # --- expert 0 weights first so the PE can start ASAP, x on a second
        # DMA queue (gpsimd) so the loads overlap the weight stream.
        w1_sb = w1pool.tile([P, KO1, H1], F32R, tag="w1", name="w1_sb_0")
        nc.sync.dma_start(w1_sb[:], w1[0])
        xs_sb = xpool.tile([P, KO1, BC], F32R, name="xs_sb")
        nc.gpsimd.dma_start(xs_sb[:], xst[:])
        w2_sb = w2pool.tile([P, KO2, H2], F32R, tag="w2", name="w2_sb_0")
        nc.sync.dma_start(w2_sb[:], w2[0])
        x0_sb = xpool.tile([P, KO1, BC], F32R, name="x0_sb")
        nc.gpsimd.dma_start(x0_sb[:], x0t[:])
        x1_sb = xpool.tile([P, KO1, BC], F32R, name="x1_sb")
        nc.gpsimd.dma_start(x1_sb[:], x1t[:])

        wg_sb = const.tile([P, KO1, GW], F32R, name="wg_sb")
        nc.gpsimd.dma_start(wg_sb[:], wg[:])
        b1_sb = const.tile([P, NE, MT1], F32, name="b1_sb")
        nc.gpsimd.dma_start(b1_sb[:], b1d[:])
        b2_sb = const.tile([1, NE, H2], F32R, name="b2_sb")
        nc.gpsimd.dma_start(b2_sb[:], b2d[:])
        ones_sb = const.tile([1, P], F32R, name="ones_sb")
        nc.gpsimd.dma_start(ones_sb[:], onesd[:])

        g_sb = const.tile([P, BT, GW], F32, name="g_sb")
        xsrc = {0: (xs_sb, 16, 12), 1: (x0_sb, 0, 8), 2: (x1_sb, 8, 8)}

        # --- gates: z = x @ Wg per B-tile, then softmax along the free dim.
        for gi in (0, 1, 2):
            src, off, w = xsrc[gi]
            for bt in range(BT):
                psz = l2ps.tile([P, w], F32, tag="l2", name=f"psz_{gi}_{bt}")
                for ko in range(KO1):
                    nc.tensor.matmul(
                        psz[:],
                        src[:, ko, bt * P:(bt + 1) * P],
                        wg_sb[:, ko, off:off + w],
                        start=(ko == 0),
                        stop=(ko == KO1 - 1),
                    )
                nmax = gtmp.tile([P, 1], F32, tag="gn", name=f"nmax_{gi}_{bt}")
                nc.vector.tensor_reduce(
                    nmax[:], psz[:], axis=mybir.AxisListType.X,
                    op=mybir.AluOpType.max, negate=True,
                )
                e_sb = gtmp.tile([P, w], F32, tag="ge", name=f"e_sb_{gi}_{bt}")
                nc.scalar.activation(e_sb[:], psz[:], EXP, bias=nmax[:], scale=1.0)
                ssum = gtmp.tile([P, 1], F32, tag="gs", name=f"ssum_{gi}_{bt}")
                nc.vector.tensor_reduce(
                    ssum[:], e_sb[:], axis=mybir.AxisListType.X, op=ADD,
                )
                rsum = gtmp.tile([P, 1], F32, tag="gr", name=f"rsum_{gi}_{bt}")
                nc.vector.reciprocal(rsum[:], ssum[:])
                nc.vector.tensor_scalar_mul(g_sb[:, bt, off:off + w], e_sb[:], rsum[:])

        # --- accumulators for the three gated combines.
        t0a = const.tile([P, BT, H2], F32, name="t0a")
        t1a = const.tile([P, BT, H2], F32, name="t1a")
        tsa = const.tile([P, BT, H2], F32, name="tsa")

        for e in range(NE):
            if e > 0:
                w1_sb = w1pool.tile([P, KO1, H1], F32R, tag="w1", name=f"w1_sb_{e}")
                nc.sync.dma_start(w1_sb[:], w1[e])
                w2_sb = w2pool.tile([P, KO2, H2], F32R, tag="w2", name=f"w2_sb_{e}")
                nc.sync.dma_start(w2_sb[:], w2[e])
            src_sb = xs_sb if e < 4 else (x0_sb if e < 8 else x1_sb)

            # layer 1: hT[H1, BC] = relu(W1[e].T-chunks @ xT + b1[e])
            hT = hpool.tile([P, MT1, BC], F32R, tag="h", name=f"hT_{e}")
            for m in range(MT1):
                ph = l1ps.tile([P, BC], F32, tag="l1", name=f"ph_{e}_{m}")
                for ko in range(KO1):
                    nc.tensor.matmul(
                        ph[:],
                        w1_sb[:, ko, m * P:(m + 1) * P],
                        src_sb[:, ko, :],
                        start=(ko == 0),
                        stop=(ko == KO1 - 1),
                    )
                nc.scalar.activation(
                    hT[:, m, :], ph[:], RELU, bias=b1_sb[:, e, m:m + 1], scale=1.0,
                )

            # layer 2 + b2 (K=1 ones-row matmul) + relu + gated accumulate
            if e < 4:
                targets = [(t0a, 0 + e), (t1a, 8 + e), (tsa, 16 + e)]
            elif e < 8:
                targets = [(t0a, 0 + e), (tsa, 16 + e)]
            else:
                targets = [(t1a, 8 + 4 + (e - 8)), (tsa, 16 + e)]
            for bt in range(BT):
                po = l2ps.tile([P, H2], F32, tag="l2", name=f"po_{e}_{bt}")
                for kh in range(KO2):
                    nc.tensor.matmul(
                        po[:],
                        hT[:, kh, bt * P:(bt + 1) * P],
                        w2_sb[:, kh, :],
                        start=(kh == 0),
                        stop=False,
                    )
                nc.tensor.matmul(
                    po[:], ones_sb[:1, :], b2_sb[:1, e, :], start=False, stop=True,
                )
                o_sb = opool.tile([P, H2], F32, tag="o", name=f"o_{e}_{bt}")
                nc.scalar.activation(o_sb[:], po[:], RELU)
                for acc, col in targets:
                    sc = g_sb[:, bt, col:col + 1]
                    if e == 0:
                        nc.vector.tensor_scalar_mul(acc[:, bt, :], o_sb[:], sc)
                    else:
                        nc.vector.scalar_tensor_tensor(
                            acc[:, bt, :], o_sb[:], sc, acc[:, bt, :],
                            op0=MULT, op1=ADD,
                        )

        nc.sync.dma_start(t0d[:], t0a[:])
        nc.sync.dma_start(t1d[:], t1a[:])
        nc.sync.dma_start(tsd[:], tsa[:])

    nc.finalize()
    return nc


_PROGRAM = None


def _get_program():
    global _PROGRAM
    if _PROGRAM is None:
        _PROGRAM = _build_program()
    return _PROGRAM


def _prep_inputs(x0, x1, xs, W1, b1, W2, b2, Wg0, Wg1, Wgs):
    """Host-side shard + relayout into the DMA-friendly per-core layouts."""
    f = np.float32

    def xt_core(x, c):
        # x [B, D] -> core slice transposed/tiled to [P, KO1, BC]
        s = np.asarray(x[c * BC:(c + 1) * BC], f).T          # [D, BC]
        return np.ascontiguousarray(s.reshape(KO1, P, BC).transpose(1, 0, 2))

    w1r = np.ascontiguousarray(
        np.asarray(W1, f).reshape(NE, KO1, P, H1).transpose(0, 2, 1, 3))
    w2r = np.ascontiguousarray(
        np.asarray(W2, f).reshape(NE, KO2, P, H2).transpose(0, 2, 1, 3))
    wgr = np.ascontiguousarray(
        np.concatenate([np.asarray(Wg0, f), np.asarray(Wg1, f), np.asarray(Wgs, f)],
                       axis=1).reshape(KO1, P, GW).transpose(1, 0, 2))
    b1r = np.ascontiguousarray(np.asarray(b1, f).reshape(NE, MT1, P).transpose(2, 0, 1))
    b2r = np.ascontiguousarray(np.asarray(b2, f).reshape(1, NE, H2))

    in_maps = []
    for c in range(NCORES):
        in_maps.append({
            "xst": xt_core(xs, c),
            "x0t": xt_core(x0, c),
            "x1t": xt_core(x1, c),
            "w1": w1r,
            "w2": w2r,
            "wg": wgr,
            "b1d": b1r,
            "b2d": b2r,
            "onesd": np.ones((1, P), f),
        })
    return in_maps


def _assemble(results):
    outs = []
    for name in ("t0d", "t1d", "tsd"):
        parts = [
            results[c][name].transpose(1, 0, 2).reshape(BC, H2)
            for c in range(NCORES)
        ]
        outs.append(np.ascontiguousarray(np.concatenate(parts, axis=0)))
    return tuple(outs)


def kernel(x0, x1, xs, W1, b1, W2, b2, Wg0, Wg1, Wgs, **run_kwargs):
    nc = _get_program()
    in_maps = _prep_inputs(x0, x1, xs, W1, b1, W2, b2, Wg0, Wg1, Wgs)
    res = run_bass_kernel_spmd(nc, in_maps, core_ids=list(range(NCORES)), **run_kwargs)
    out = _assemble(res.results)
    if run_kwargs:
        return out, res
    return out


# revision 10
# speedup vs baseline: 1.4213x; 1.4213x over previous
"""CGC multi-gate MoE kernel for Trainium2 (8 NeuronCores, data-parallel over batch).

Problem: 12 experts (4 shared / 4 task0 / 4 task1), each a 2-layer ReLU MLP
D=1024 -> H1=512 -> H2=256, over B=4096 rows; 3 softmax gates combine expert
outputs into t0/t1/ts [B, 256].

Strategy: pure batch data-parallel (512 rows/core, no collectives). Host
pre-transposes x (so the contraction dim D lands on SBUF partitions) and
pre-tiles weights into the exact SBUF layout for fully-contiguous DMA.
Layer-1 output is kept transposed on-chip ([H1, B] layout) so layer 2 needs no
transpose and its output lands with B on partitions, ready for the per-row
gate-weighted combine on DVE. Biases: b1 fused into the layer-1 ReLU copy on
ACT (per-partition bias); b2 added in-PSUM via a K=1 ones-row matmul.

Scheduling: x loads stream per-K-chunk on the vector engine's DMA queue while
weights stream on sync's; expert order is task0 -> shared -> task1 so the
first expert only needs x0 (first x to arrive) and every gate's operands are
resident before its matmuls come up in the PE stream.

Matmul dtype is selectable: float32r (~2e-4 rel err) or bfloat16 (~3e-3,
faster weight loads + dual-pumped moving operand + half the DMA traffic).
"""
import os
import sys

for _p in ("/opt/trn_rl_repo", "/root/.axon_site/_ro/trn_rl_repo"):
    if os.path.isdir(_p):
        if _p not in sys.path:
            sys.path.insert(0, _p)
        break

import numpy as np
from contextlib import ExitStack

import ml_dtypes

import concourse.bass as bass
import concourse.mybir as mybir
import concourse.tile as tile
from concourse import bacc
from concourse.bass_utils import run_bass_kernel_spmd

B, D, H1, H2 = 4096, 1024, 512, 256
NE = 12          # 4 shared + 4 task0 + 4 task1
NCORES = 8
BC = B // NCORES # 512 rows per core
P = 128
KO1 = D // P     # 8 contraction chunks, layer 1
KO2 = H1 // P    # 4 contraction chunks, layer 2
MT1 = H1 // P    # 4 output M-tiles, layer 1 (H1 on partitions)
BT = BC // P     # 4 B-tiles per core
GW = 28          # gate logit widths, concatenated: 8 (g0) + 8 (g1) + 12 (gs)

F32 = mybir.dt.float32
RELU = mybir.ActivationFunctionType.Relu
EXP = mybir.ActivationFunctionType.Exp
MULT = mybir.AluOpType.mult
ADD = mybir.AluOpType.add

# expert processing order: task0 (needs x0, first to arrive), shared, task1
EXPERT_ORDER = [4, 5, 6, 7, 0, 1, 2, 3, 8, 9, 10, 11]


def _build_program(use_bf16):
    MMD = mybir.dt.bfloat16 if use_bf16 else mybir.dt.float32r
    nc = bacc.Bacc("TRN2", target_bir_lowering=False, debug=False, num_devices=NCORES)

    xst = nc.dram_tensor("xst", [P, KO1, BC], MMD, kind="ExternalInput")
    x0t = nc.dram_tensor("x0t", [P, KO1, BC], MMD, kind="ExternalInput")
    x1t = nc.dram_tensor("x1t", [P, KO1, BC], MMD, kind="ExternalInput")
    w1 = nc.dram_tensor("w1", [NE, P, KO1, H1], MMD, kind="ExternalInput")
    w2 = nc.dram_tensor("w2", [NE, P, KO2, H2], MMD, kind="ExternalInput")
    wg = nc.dram_tensor("wg", [P, KO1, GW], MMD, kind="ExternalInput")
    b1d = nc.dram_tensor("b1d", [P, NE, MT1], F32, kind="ExternalInput")
    b2d = nc.dram_tensor("b2d", [1, NE, H2], MMD, kind="ExternalInput")
    onesd = nc.dram_tensor("onesd", [1, P], MMD, kind="ExternalInput")
    t0d = nc.dram_tensor("t0d", [P, BT, H2], F32, kind="ExternalOutput")
    t1d = nc.dram_tensor("t1d", [P, BT, H2], F32, kind="ExternalOutput")
    tsd = nc.dram_tensor("tsd", [P, BT, H2], F32, kind="ExternalOutput")

    with tile.TileContext(nc) as tc, ExitStack() as ctx:
        const = ctx.enter_context(tc.tile_pool(name="const", bufs=1))
        xpool = ctx.enter_context(tc.tile_pool(name="xpool", bufs=1))
        w1pool = ctx.enter_context(tc.tile_pool(name="w1pool", bufs=2))
        w2pool = ctx.enter_context(tc.tile_pool(name="w2pool", bufs=2))
        hpool = ctx.enter_context(tc.tile_pool(name="hpool", bufs=2))
        opool = ctx.enter_context(tc.tile_pool(name="opool", bufs=8))
        gtmp = ctx.enter_context(tc.tile_pool(name="gtmp", bufs=2))
        l1ps = ctx.enter_context(tc.tile_pool(name="l1ps", bufs=4, space="PSUM"))
        l2ps = ctx.enter_context(tc.tile_pool(name="l2ps", bufs=4, space="PSUM"))

        # tiny constants on the scalar engine's DMA queue (idle at start)
        wg_sb = const.tile([P, KO1, GW], MMD, name="wg_sb")
        nc.scalar.dma_start(wg_sb[:], wg[:])
        b1_sb = const.tile([P, NE, MT1], F32, name="b1_sb")
        nc.scalar.dma_start(b1_sb[:], b1d[:])
        b2_sb = const.tile([1, NE, H2], MMD, name="b2_sb")
        nc.scalar.dma_start(b2_sb[:], b2d[:])
        ones_sb = const.tile([1, P], MMD, name="ones_sb")
        nc.scalar.dma_start(ones_sb[:], onesd[:])

        # x loads, chunked per K-slice on the vector engine's DMA queue, in
        # expert-consumption order: x0 first, xs second, x1 last.
        x0_sb = xpool.tile([P, KO1, BC], MMD, name="x0_sb")
        xs_sb = xpool.tile([P, KO1, BC], MMD, name="xs_sb")
        x1_sb = xpool.tile([P, KO1, BC], MMD, name="x1_sb")
        for ko in range(KO1):
            nc.scalar.dma_start(x0_sb[:, ko, :], x0t[:, ko, :])
        nc.scalar.dma_start(xs_sb[:], xst[:])
        nc.scalar.dma_start(x1_sb[:], x1t[:])

        g_sb = const.tile([P, BT, GW], F32, name="g_sb")
        t0a = const.tile([P, BT, H2], F32, name="t0a")
        t1a = const.tile([P, BT, H2], F32, name="t1a")
        tsa = const.tile([P, BT, H2], F32, name="tsa")
        acc_first = {id(t0a): True, id(t1a): True, id(tsa): True}

        def emit_gate(src_sb, off, w):
            for bt in range(BT):
                psz = l2ps.tile([P, w], F32, tag="l2", name=f"psz_{off}_{bt}")
                for ko in range(KO1):
                    nc.tensor.matmul(
                        psz[:],
                        src_sb[:, ko, bt * P:(bt + 1) * P],
                        wg_sb[:, ko, off:off + w],
                        start=(ko == 0),
                        stop=(ko == KO1 - 1),
                    )
                nmax = gtmp.tile([P, 1], F32, tag="gn", name=f"nmax_{off}_{bt}")
                nc.vector.tensor_reduce(
                    nmax[:], psz[:], axis=mybir.AxisListType.X,
                    op=mybir.AluOpType.max, negate=True,
                )
                e_sb = gtmp.tile([P, w], F32, tag="ge", name=f"e_sb_{off}_{bt}")
                nc.scalar.activation(e_sb[:], psz[:], EXP, bias=nmax[:], scale=1.0)
                ssum = gtmp.tile([P, 1], F32, tag="gs", name=f"ssum_{off}_{bt}")
                nc.vector.tensor_reduce(
                    ssum[:], e_sb[:], axis=mybir.AxisListType.X, op=ADD,
                )
                rsum = gtmp.tile([P, 1], F32, tag="gr", name=f"rsum_{off}_{bt}")
                nc.vector.reciprocal(rsum[:], ssum[:])
                nc.vector.tensor_scalar_mul(g_sb[:, bt, off:off + w], e_sb[:], rsum[:])

        for idx, e in enumerate(EXPERT_ORDER):
            src_sb = xs_sb if e < 4 else (x0_sb if e < 8 else x1_sb)

            w1_sb = w1pool.tile([P, KO1, H1], MMD, tag="w1", name=f"w1_sb_{e}")
            if idx == 0:
                # chunked so the first matmul can start before the full tile lands
                for ko in range(KO1):
                    nc.sync.dma_start(w1_sb[:, ko, :], w1[e, :, ko, :])
            else:
                nc.sync.dma_start(w1_sb[:], w1[e])
            w2_sb = w2pool.tile([P, KO2, H2], MMD, tag="w2", name=f"w2_sb_{e}")
            nc.sync.dma_start(w2_sb[:], w2[e])

            # layer 1: hT[H1, BC] = relu(W1[e].T-chunks @ xT + b1[e])
            hT = hpool.tile([P, MT1, BC], MMD, tag="h", name=f"hT_{e}")
            for m in range(MT1):
                ph = l1ps.tile([P, BC], F32, tag="l1", name=f"ph_{e}_{m}")
                for ko in range(KO1):
                    nc.tensor.matmul(
                        ph[:],
                        w1_sb[:, ko, m * P:(m + 1) * P],
                        src_sb[:, ko, :],
                        start=(ko == 0),
                        stop=(ko == KO1 - 1),
                    )
                nc.scalar.activation(
                    hT[:, m, :], ph[:], RELU, bias=b1_sb[:, e, m:m + 1], scale=1.0,
                )

            # gates go into the PE stream right when their operands are resident
            if idx == 0:
                emit_gate(x0_sb, 0, 8)    # g0
                emit_gate(xs_sb, 16, 12)  # gs
            elif idx == 1:
                emit_gate(x1_sb, 8, 8)    # g1

            # layer 2 + b2 (K=1 ones-row matmul) + relu + gated accumulate
            if e < 4:
                targets = [(t0a, 0 + e), (t1a, 8 + e), (tsa, 16 + e)]
            elif e < 8:
                targets = [(t0a, 0 + e), (tsa, 16 + e)]
            else:
                targets = [(t1a, 8 + 4 + (e - 8)), (tsa, 16 + e)]
            for bt in range(BT):
                po = l2ps.tile([P, H2], F32, tag="l2", name=f"po_{e}_{bt}")
                for kh in range(KO2):
                    nc.tensor.matmul(
                        po[:],
                        hT[:, kh, bt * P:(bt + 1) * P],
                        w2_sb[:, kh, :],
                        start=(kh == 0),
                        stop=False,
                    )
                nc.tensor.matmul(
                    po[:], ones_sb[:1, :], b2_sb[:1, e, :], start=False, stop=True,
                )
                o_sb = opool.tile([P, H2], F32, tag="o", name=f"o_{e}_{bt}")
                nc.scalar.activation(o_sb[:], po[:], RELU)
                for acc, col in targets:
                    sc = g_sb[:, bt, col:col + 1]
                    if acc_first[id(acc)]:
                        nc.vector.tensor_scalar_mul(acc[:, bt, :], o_sb[:], sc)
                    else:
                        nc.vector.scalar_tensor_tensor(
                            acc[:, bt, :], o_sb[:], sc, acc[:, bt, :],
                            op0=MULT, op1=ADD,
                        )
            for acc, _ in targets:
                acc_first[id(acc)] = False

            if idx == 7:
                # t0 got its last contribution (order: e4..7 then e0..3)
                nc.sync.dma_start(t0d[:], t0a[:])

        nc.sync.dma_start(t1d[:], t1a[:])
        nc.sync.dma_start(tsd[:], tsa[:])

    nc.finalize()
    return nc


_PROGRAMS = {}


def _get_program(use_bf16):
    if use_bf16 not in _PROGRAMS:
        _PROGRAMS[use_bf16] = _build_program(use_bf16)
    return _PROGRAMS[use_bf16]


def _prep_inputs(x0, x1, xs, W1, b1, W2, b2, Wg0, Wg1, Wgs, use_bf16):
    """Host-side shard + relayout into the DMA-friendly per-core layouts."""
    f = np.float32
    md = ml_dtypes.bfloat16 if use_bf16 else np.float32

    def xt_core(x, c):
        # x [B, D] -> core slice transposed/tiled to [P, KO1, BC]
        s = np.asarray(x[c * BC:(c + 1) * BC], f).T          # [D, BC]
        return np.ascontiguousarray(
            s.reshape(KO1, P, BC).transpose(1, 0, 2).astype(md))

    w1r = np.ascontiguousarray(
        np.asarray(W1, f).reshape(NE, KO1, P, H1).transpose(0, 2, 1, 3).astype(md))
    w2r = np.ascontiguousarray(
        np.asarray(W2, f).reshape(NE, KO2, P, H2).transpose(0, 2, 1, 3).astype(md))
    wgr = np.ascontiguousarray(
        np.concatenate([np.asarray(Wg0, f), np.asarray(Wg1, f), np.asarray(Wgs, f)],
                       axis=1).reshape(KO1, P, GW).transpose(1, 0, 2).astype(md))
    b1r = np.ascontiguousarray(np.asarray(b1, f).reshape(NE, MT1, P).transpose(2, 0, 1))
    b2r = np.ascontiguousarray(np.asarray(b2, f).reshape(1, NE, H2).astype(md))
    ones = np.ones((1, P), md)

    in_maps = []
    for c in range(NCORES):
        in_maps.append({
            "xst": xt_core(xs, c),
            "x0t": xt_core(x0, c),
            "x1t": xt_core(x1, c),
            "w1": w1r,
            "w2": w2r,
            "wg": wgr,
            "b1d": b1r,
            "b2d": b2r,
            "onesd": ones,
        })
    return in_maps


def _assemble(results):
    outs = []
    for name in ("t0d", "t1d", "tsd"):
        parts = [
            results[c][name].transpose(1, 0, 2).reshape(BC, H2)
            for c in range(NCORES)
        ]
        outs.append(np.ascontiguousarray(np.concatenate(parts, axis=0)))
    return tuple(outs)


def kernel(x0, x1, xs, W1, b1, W2, b2, Wg0, Wg1, Wgs, use_bf16=False, **run_kwargs):
    nc = _get_program(use_bf16)
    in_maps = _prep_inputs(x0, x1, xs, W1, b1, W2, b2, Wg0, Wg1, Wgs, use_bf16)
    res = run_bass_kernel_spmd(nc, in_maps, core_ids=list(range(NCORES)), **run_kwargs)
    out = _assemble(res.results)
    if run_kwargs:
        return out, res
    return out
